# revision 8
# baseline (speedup 1.0000x reference)
"""Deep ReLU RNN (B=32, S=2048, I=256, H=512, L=4) on 8 Trainium2 cores.

Strategy: layer-pipeline wavefront. Cores 4p+l (p = batch half, l = layer)
each own one layer's weights and scan the sequence for their 16-sample batch
half. Sequence is cut into chunks of Tc steps; core (p, l) processes chunk
c = i - 2l at wavefront iteration i. Chunk handoff between consecutive
layers rides a per-iteration 4-rank AllGather (groups [[0..3],[4..7]]).

Everything stays in [H, B] (feature-on-partition) layout so the recurrent
matmul h_new^T = Whh^T.T @ h^T needs no per-step transpose:
  lhsT (stationary) = Whh^T chunk [128 k, 128 j] bf16 (FWL weight loads)
  rhs  (moving)     = h^T chunk   [128 k, 16 b]
The input GEMM (pre = x @ Wih^T + biases) is done per chunk with N=512
moving tiles into the spare 4 PSUM banks; biases fold into the PSUM->SBUF
copy on the Scalar engine (per-partition bias). Per scan step:
4 j-chunks x 4 k-chunks matmuls accumulate into 4 PSUM banks, then
DVE add(pre) + ACT relu in two half-groups so the next step's first
matmuls overlap the second half's epilogue.
"""

import os
import sys

sys.path.insert(0, "/opt/trn_rl_repo")

import numpy as np
import ml_dtypes

import concourse.bass as bass
import concourse.bacc as bacc
import concourse.mybir as mybir
import concourse.tile as tile
from concourse.bass_utils import run_bass_kernel_spmd

BF16 = mybir.dt.bfloat16
F32 = mybir.dt.float32

N_CORES = 8
L = 4
H = 512
I_IN = 256
JC = 4          # H / 128 output-feature chunks
KC = 4          # H / 128 contraction chunks (layer-0 input padded 256->512)
BH = 16         # batch half per pipeline


class Cfg:
    def __init__(self, S, Tc, B=32):
        assert S % Tc == 0 and Tc * BH % 512 == 0
        self.S = S
        self.Tc = Tc
        self.B = B
        self.NC = S // Tc               # number of sequence chunks
        self.NI = self.NC + 2 * (L - 1)  # wavefront iterations (skew 2 per layer)
        self.TB = Tc * BH // 512        # 512-wide column blocks per chunk
        self.cols = Tc * BH             # (t, b) columns per chunk per j-chunk


class TC(tile.TileContext):
    """TileContext whose exit drain spills its sem waits onto NOPs.

    walrus rejects instructions carrying more than one sync wait; the stock
    tail drain waits on every proc sem the kernel touched.
    """

    def _drain_and_barrier(self, tick_clock, wait_clock):
        from concourse.tile import ScopedClock

        spills = [self.nc.sync.nop(nofuse=True) for _ in range(32)]
        drain_inst = self.nc.sync.drain()
        wait_clock.add_sem_waits(
            drain_inst.ins, ScopedClock({None: tick_clock.global_clock})
        )
        si = drain_inst.ins.sync_info
        if si is not None and len(si.on_wait) > 1:
            waits = list(si.on_wait)
            rest, keep = waits[:-1], waits[-1:]
            drain_inst.ins.sync_info = mybir.SyncInfo(
                on_wait=keep, on_update=list(si.on_update)
            )
            assert len(rest) <= len(spills), "too many tail waits"
            for k, w in enumerate(rest):
                spills[k].ins.sync_info = mybir.SyncInfo(on_wait=[w], on_update=[])

        self.nc.all_engine_barrier()
        assert self.sems is not None
        popped = self.nc._tile_sem_poison_stack.pop()
        assert popped is self._sem_poison
        self.nc.clear_and_free_semaphores(list(self.sems.allocated().values()))
        self.nc.all_engine_barrier()


def build(cfg: Cfg) -> bass.Bass:
    Tc, NI, NC, TB = cfg.Tc, cfg.NI, cfg.NC, cfg.TB
    nc = bacc.Bacc("TRN2", target_bir_lowering=False, debug=False, num_devices=N_CORES)

    w_pre_d = nc.dram_tensor("w_pre", [128, KC, JC, 128], BF16, kind="ExternalInput")
    w_scan_d = nc.dram_tensor("w_scan", [128, KC, JC, 128], BF16, kind="ExternalInput")
    bias_d = nc.dram_tensor("bias", [128, JC], F32, kind="ExternalInput")
    hscale_d = nc.dram_tensor("hscale", [128, NI], F32, kind="ExternalInput")
    selc_d = nc.dram_tensor("selc", [128, 3], F32, kind="ExternalInput")
    h0m_d = nc.dram_tensor("h0m", [NI, 128, JC * BH], BF16, kind="ExternalInput")
    xin_d = nc.dram_tensor("xin", [NI, 128, KC, Tc, BH], BF16, kind="ExternalInput")
    ident_d = nc.dram_tensor("ident", [128, 128], BF16, kind="ExternalInput")

    x_out_d = nc.dram_tensor("x_out", [NC, 128, JC, Tc, BH], F32, kind="ExternalOutput")
    h_fin_d = nc.dram_tensor("h_fin", [NI, 128, JC, BH], F32, kind="ExternalOutput")

    with tile.TileContext(nc, num_cores=N_CORES) as tc:
        with (
            tc.tile_pool(name="const", bufs=1) as constp,
            tc.tile_pool(name="sb", bufs=2) as sb,
            tc.tile_pool(name="shardp", bufs=6) as shardp,
            tc.tile_pool(name="tmpp", bufs=4) as tmpp,
            tc.tile_pool(name="pscan", bufs=1, space="PSUM") as pscan,
            tc.tile_pool(name="ppre", bufs=2, space="PSUM") as ppre,
            tc.tile_pool(name="dram", bufs=2, space="DRAM") as dram,
        ):
            # ---- constants, loaded once ----
            w_pre = constp.tile([128, KC, JC, 128], BF16, name="w_pre_sb")
            w_scan = constp.tile([128, KC, JC, 128], BF16, name="w_scan_sb")
            bias = constp.tile([128, JC], F32, name="bias_sb")
            hscale = constp.tile([128, NI], F32, name="hscale_sb")
            selc = constp.tile([128, 3], F32, name="selc_sb")
            h0m = constp.tile([128, NI, JC * BH], BF16, name="h0m_sb")
            ident = constp.tile([128, 128], BF16, name="ident_sb")
            nc.sync.dma_start(ident[:], ident_d[:])
            nc.sync.dma_start(w_pre[:], w_pre_d[:])
            nc.sync.dma_start(w_scan[:], w_scan_d[:])
            nc.sync.dma_start(bias[:], bias_d[:])
            nc.sync.dma_start(hscale[:], hscale_d[:])
            nc.sync.dma_start(selc[:], selc_d[:])
            nc.sync.dma_start(h0m[:], h0m_d.rearrange("i p c -> p i c"))

            psum_scan = pscan.tile([128, JC, 512], F32, name="psum_scan")

            prev_h = None      # previous iteration's h_hist tile
            prev_ag_out = None

            for i in range(NI):
                # ---- ship last iteration's chunk via AllGather ----
                ag_in = dram.tile([128, JC * cfg.cols], BF16, name="ag_in", tag="ag_in")
                ag_out = dram.tile(
                    [4 * 128, JC * cfg.cols], BF16, name="ag_out", tag="ag_out"
                )
                if prev_h is not None:
                    nc.gpsimd.dma_start(ag_in[:], prev_h.rearrange("p a t b -> p (a t b)"))
                else:
                    seed = sb.tile([128, JC, Tc, BH], BF16, name="seed", tag="h_hist")
                    nc.vector.memset(seed[:], 0.0)
                    prev_h = seed
                    nc.gpsimd.dma_start(ag_in[:], prev_h.rearrange("p a t b -> p (a t b)"))
                nc.gpsimd.collective_compute(
                    "AllGather",
                    mybir.AluOpType.bypass,
                    ins=[ag_in[:]],
                    outs=[ag_out[:]],
                    replica_groups=[[0, 1, 2, 3], [4, 5, 6, 7]],
                )

                # ---- assemble x for my chunk: select(shards of AG(i-1)) + xin ----
                xin_sb = sb.tile([128, KC, Tc, BH], BF16, name="xin_sb", tag="xin_sb")
                nc.sync.dma_start(xin_sb[:], xin_d[i])
                x_eff = sb.tile([128, KC, Tc, BH], BF16, name="x_eff", tag="x_eff")
                if prev_ag_out is None:
                    nc.vector.tensor_copy(x_eff[:], xin_sb[:])
                else:
                    for s in range(3):
                        sh = shardp.tile(
                            [128, KC, Tc, BH], BF16, name=f"sh{s}", tag="shard"
                        )
                        nc.sync.dma_start(
                            sh.rearrange("p a t b -> p (a t b)"),
                            prev_ag_out[128 * s:128 * (s + 1), :],
                        )
                        nc.vector.scalar_tensor_tensor(
                            out=x_eff[:],
                            in0=sh[:],
                            scalar=selc[:, s:s + 1],
                            in1=xin_sb[:] if s == 0 else x_eff[:],
                            op0=mybir.AluOpType.mult,
                            op1=mybir.AluOpType.add,
                        )
                prev_ag_out = ag_out

                # ---- pre-GEMM: pre = Wih^T.T @ x_eff (+ biases via ACT copy) ----
                psum_pre = ppre.tile([128, TB, 512], F32, name="psum_pre", tag="ppre")
                pre_sb = sb.tile([128, JC, Tc, BH], BF16, name="pre_sb", tag="pre_sb")
                x_cols = x_eff.rearrange("p a t b -> p a (t b)")
                pre_cols = pre_sb.rearrange("p a t b -> p a (t b)")
                for tb in range(TB):
                    for jc in range(JC):
                        for kc in range(KC):
                            nc.tensor.matmul(
                                psum_pre[:, tb, :],
                                w_pre[:, kc, jc, :],
                                x_cols[:, kc, tb * 512:(tb + 1) * 512],
                                start=(kc == 0),
                                stop=(kc == KC - 1),
                            )
                        nc.scalar.activation(
                            pre_cols[:, jc, tb * 512:(tb + 1) * 512],
                            psum_pre[:, tb, :],
                            mybir.ActivationFunctionType.Identity,
                            bias=bias[:, jc:jc + 1],
                            scale=1.0,
                        )

                # ---- chunk-boundary h (+h0 injection on my chunk 0) ----
                h_bound = tmpp.tile([128, JC * BH], BF16, name="h_bound", tag="h_bound")
                nc.vector.scalar_tensor_tensor(
                    out=h_bound[:],
                    in0=prev_h[:, :, Tc - 1, :],
                    scalar=hscale[:, i:i + 1],
                    in1=h0m[:, i, :],
                    op0=mybir.AluOpType.mult,
                    op1=mybir.AluOpType.add,
                )

                # ---- scan Tc steps ----
                h_hist = sb.tile([128, JC, Tc, BH], BF16, name="h_hist", tag="h_hist")
                for t in range(Tc):
                    for half in range(2):
                        jcs = (0, 1) if half == 0 else (2, 3)
                        for jc in jcs:
                            # inject pre_t into the accumulator, then Whh chunks
                            nc.tensor.matmul(
                                psum_scan[:, jc, 0:BH],
                                ident[:],
                                pre_sb[:, jc, t, :],
                                start=True,
                                stop=False,
                            )
                            for kc in range(KC):
                                rhs = (
                                    h_bound[:, kc * BH:(kc + 1) * BH]
                                    if t == 0
                                    else h_hist[:, kc, t - 1, :]
                                )
                                nc.tensor.matmul(
                                    psum_scan[:, jc, 0:BH],
                                    w_scan[:, kc, jc, :],
                                    rhs,
                                    start=False,
                                    stop=(kc == KC - 1),
                                )
                        lo = jcs[0]
                        nc.vector.tensor_scalar_max(
                            h_hist[:, lo:lo + 2, t, :],
                            psum_scan[:, lo:lo + 2, 0:BH],
                            0.0,
                        )

                # ---- outputs ----
                if i >= 2 * (L - 1):
                    nc.gpsimd.dma_start(x_out_d[i - 2 * (L - 1)], h_hist[:])
                nc.gpsimd.dma_start(h_fin_d[i], h_hist[:, :, Tc - 1, :])
                prev_h = h_hist

    nc.finalize()
    return nc


# ---------------- host side ----------------

def _chunk_w(w):
    """[H_out, K] (j, k) -> [128, KC, JC, 128] lhsT chunks, zero-padding K to 512."""
    Hout, K = w.shape
    wp = np.zeros((512, 512), np.float32)
    wp[:Hout, :K] = w
    out = np.zeros((128, KC, JC, 128), np.float32)
    for kc in range(KC):
        for jc in range(JC):
            # lhsT[k_local, j_local] = w[j, k]
            out[:, kc, jc, :] = wp[jc * 128:(jc + 1) * 128, kc * 128:(kc + 1) * 128].T
    return out.astype(ml_dtypes.bfloat16)


_BUILD_CACHE = {}


def _get_nc(cfg: Cfg):
    key = (cfg.S, cfg.Tc)
    if key not in _BUILD_CACHE:
        _BUILD_CACHE[key] = build(cfg)
    return _BUILD_CACHE[key]


def run(cfg: Cfg, inputs, h0, w_ih_0, w_ih_n, w_hh, b_ih, b_hh):
    S, Tc, NI, NC = cfg.S, cfg.Tc, cfg.NI, cfg.NC
    B = inputs.shape[0]
    bf = ml_dtypes.bfloat16

    nc = _get_nc(cfg)

    in_maps = []
    zero_xin = np.zeros((NI, 128, KC, Tc, BH), bf)
    for core in range(N_CORES):
        p, l = divmod(core, 4)
        w_ih = w_ih_0 if l == 0 else w_ih_n[l - 1]
        w_pre = _chunk_w(np.asarray(w_ih, np.float32))
        w_scan = _chunk_w(np.asarray(w_hh[l], np.float32))
        bias = (np.asarray(b_ih[l], np.float32) + np.asarray(b_hh[l], np.float32))
        bias_c = np.ascontiguousarray(
            np.broadcast_to(bias.reshape(JC, 128).T, (128, JC))
        ).astype(np.float32)

        hscale = np.ones((128, NI), np.float32)
        hscale[:, 2 * l] = 0.0

        selc = np.zeros((128, 3), np.float32)
        if l >= 1:
            selc[:, l - 1] = 1.0

        h0m = np.zeros((NI, 128, JC * BH), np.float32)
        # h0 chunk for this core at its chunk-0 iteration: [j_local, (jc, b)]
        h0_t = np.asarray(h0[l][p * BH:(p + 1) * BH], np.float32)  # [BH, H]
        h0_r = h0_t.T.reshape(JC, 128, BH).transpose(1, 0, 2).reshape(128, JC * BH)
        h0m[2 * l] = h0_r

        if l == 0:
            xin = np.zeros((NI, 128, KC, Tc, BH), np.float32)
            xb = np.asarray(inputs[p * BH:(p + 1) * BH], np.float32)  # [BH, S, I]
            # xin[i, k_local, kc, t, b] = x[b, i*Tc + t, kc*128 + k_local]
            xt = xb.transpose(2, 1, 0).reshape(I_IN, NC, Tc, BH)  # [I, c, t, b]
            xin[:NC, :, :I_IN // 128] = xt.reshape(2, 128, NC, Tc, BH).transpose(
                2, 1, 0, 3, 4
            )
            xin_c = xin.astype(bf)
        else:
            xin_c = zero_xin

        in_maps.append(
            {
                "ident": np.eye(128, dtype=np.float32).astype(ml_dtypes.bfloat16),
                "w_pre": w_pre,
                "w_scan": w_scan,
                "bias": bias_c,
                "hscale": hscale,
                "selc": selc,
                "h0m": h0m.astype(bf),
                "xin": xin_c,
            }
        )

    res = run_bass_kernel_spmd(
        nc,
        in_maps,
        core_ids=list(range(N_CORES)),
        trace=bool(int(os.environ.get("RNN_TRACE", "0"))),
    )

    x = np.empty((B, S, H), np.float32)
    hf = np.empty((L, B, H), np.float32)
    for p in range(2):
        xo = res.results[4 * p + 3]["x_out"]  # [NC, 128, JC, Tc, BH]
        x[p * BH:(p + 1) * BH] = (
            xo.transpose(4, 0, 3, 2, 1).reshape(BH, S, H)
        )
        for l in range(L):
            hfin = res.results[4 * p + l]["h_fin"][NC - 1 + 2 * l]  # [128, JC, BH]
            hf[l, p * BH:(p + 1) * BH] = (
                hfin.transpose(2, 1, 0).reshape(BH, H)
            )
    return x, hf, res


def kernel(inputs, h0, w_ih_0, w_ih_n, w_hh, b_ih, b_hh):
    S = int(inputs.shape[1])
    Tc = 64 if S % 64 == 0 and S >= 64 else 32
    cfg = Cfg(S, Tc, B=int(inputs.shape[0]))
    x, hf, _ = run(cfg, inputs, h0, w_ih_0, w_ih_n, w_hh, b_ih, b_hh)
    return x, hf
